# revision 1
# baseline (speedup 1.0000x reference)
import sys

sys.path.insert(0, "/opt/trn_rl_repo")
import numpy as np
import concourse.mybir as mybir
from concourse import bacc
from concourse.tile import TileContext

C = 192
HEADS = 8
D = C // HEADS  # 24
N = 4096
NT = 8  # n tiles of 512
MB = 32  # m blocks of 128
EPS = 1e-5
TAPS = [(dy, dx) for dy in (-1, 0, 1) for dx in (-1, 0, 1)]
CENTER = TAPS.index((0, 0))

f32 = mybir.dt.float32
f32r = mybir.dt.float32r
f16 = mybir.dt.float16

_cache = {}


def _cast(a, dtype):
    """fp16<->f32 cast; torch's parallel kernels are ~7x faster than numpy
    (bit-identical round-to-nearest-even). Falls back to numpy."""
    try:
        import torch

        t = torch.from_numpy(np.ascontiguousarray(a))
        t = t.half() if dtype == np.float16 else t.float()
        return t.numpy()
    except Exception:
        return a.astype(dtype)


def _build_program():
    nc = bacc.Bacc("TRN2", target_bir_lowering=False, debug=False, num_devices=8)
    # channel-sharded raw x: core c holds channels 24c..24c+24, all pixels
    x_d = nc.dram_tensor("x", [D, N], f16, kind="ExternalInput").ap()
    # fused (1x1 conv) x (depthwise 3x3): per section s (q/k/v), per tap t,
    # lhsT[c, o] = w_qkv[sec_o, c] * w_dw[sec_o, tap]
    wq_d = nc.dram_tensor("wq", [C, 27 * D], f32, kind="ExternalInput").ap()
    dw_d = nc.dram_tensor("dw", [D, 3], f32, kind="ExternalInput").ap()  # biases
    wp_d = nc.dram_tensor("wp", [D + 1, C], f32, kind="ExternalInput").ap()
    gb_d = nc.dram_tensor("gb", [C, 2], f32, kind="ExternalInput").ap()
    tp_d = nc.dram_tensor("tp", [1, 1], f32, kind="ExternalInput").ap()
    id_d = nc.dram_tensor("id24", [D, D], f32, kind="ExternalInput").ap()
    # channel-sharded output: core c holds channels 24c..24c+24, all pixels
    y_d = nc.dram_tensor("y", [D, N], f16, kind="ExternalOutput").ap()

    RG = [list(range(8))]

    with TileContext(nc) as tc:
        with (
            tc.tile_pool(name="persist", bufs=1) as pp,
            tc.tile_pool(name="fb", bufs=1) as fb,
            tc.tile_pool(name="sb", bufs=2) as sb,
            tc.tile_pool(name="fp", bufs=2, space="PSUM") as fpp,
            tc.tile_pool(name="sp", bufs=1, space="PSUM") as spp,
            tc.tile_pool(name="ac", bufs=2, space="PSUM") as acp,
            tc.tile_pool(name="dram", bufs=1, space="DRAM") as dp,
        ):
            # ---- persistent sbuf tiles ----
            x16 = pp.tile([D, N], f16, tag="x16")
            xh_a = pp.tile([128, N], f16, tag="xha")  # gathered raw x rows 0:128
            xh_b = pp.tile([64, N], f16, tag="xhb")  # gathered raw x rows 128:192
            xf_a = pp.tile([128, N], f32, tag="xfa")  # x -> x_ln (in place)
            xf_b = pp.tile([64, N], f32, tag="xfb")
            wq_a = pp.tile([128, 27 * D], f32, tag="wqa")
            wq_b = pp.tile([64, 27 * D], f32, tag="wqb")
            dw_s = pp.tile([D, 3], f32, tag="dw")
            wp_s = pp.tile([D + 1, C], f32, tag="wp")
            gb_a = pp.tile([128, 2], f32, tag="gba")
            gb_b = pp.tile([64, 2], f32, tag="gbb")
            tpb = pp.tile([128, 1], f32, tag="tp")
            id_s = pp.tile([D, D], f32, tag="id")
            ones_c = pp.tile([128, 1], f32, tag="onc")  # lhsT for partition-sum
            ones_r = pp.tile([1, 128], f32, tag="onr")  # lhsT for broadcast
            q_s = pp.tile([D, N], f32r, tag="qs")
            k_s = pp.tile([D, N], f32r, tag="ks")
            v_s = pp.tile([D, N], f32, tag="vs")
            vt_s = pp.tile([128, MB * (D + 1)], f32r, tag="vt")
            y_a = pp.tile([128, N], f32, tag="ya")
            y_b = pp.tile([64, N], f32, tag="yb")

            # dram bounce buffers for collectives
            ag_in = dp.tile([D, N], f16, tag="agin")
            ag_out = dp.tile([C, N], f16, tag="agout")
            rs_in = dp.tile([C, N], f32, tag="rsin")
            rs_out = dp.tile([D, N], f32, tag="rsout")

            # ---- load inputs/weights ----
            nc.sync.dma_start(out=x16[:], in_=x_d[:])
            nc.sync.dma_start(out=wq_a[:], in_=wq_d[0:128, :])
            nc.sync.dma_start(out=wq_b[:], in_=wq_d[128:C, :])
            nc.sync.dma_start(out=dw_s[:], in_=dw_d[:])
            nc.sync.dma_start(out=wp_s[:], in_=wp_d[:])
            nc.sync.dma_start(out=gb_a[:], in_=gb_d[0:128, :])
            nc.sync.dma_start(out=gb_b[:], in_=gb_d[128:C, :])
            nc.sync.dma_start(out=tpb[:], in_=tp_d.to_broadcast([128, 1]))
            nc.sync.dma_start(out=id_s[:], in_=id_d[:])
            nc.vector.memset(ones_c[:], 1.0)
            nc.vector.memset(ones_r[:], 1.0)

            # ---- AllGather raw x (fp16) across cores ----
            nc.gpsimd.dma_start(out=ag_in[:], in_=x16[:])
            nc.gpsimd.collective_compute(
                "AllGather",
                mybir.AluOpType.bypass,
                replica_groups=RG,
                ins=[ag_in.opt()],
                outs=[ag_out.opt()],
            )
            nc.sync.dma_start(out=xh_a[:], in_=ag_out[0:128, :])
            nc.sync.dma_start(out=xh_b[:], in_=ag_out[128:C, :])
            nc.vector.tensor_copy(xf_a[:], xh_a[:])
            nc.vector.tensor_copy(xf_b[:], xh_b[:])

            # ---- LayerNorm over channel dim, tiled by 512 pixels ----
            for j in range(NT):
                sl = slice(j * 512, (j + 1) * 512)
                s1 = fpp.tile([1, 512], f32, tag="fp")
                nc.tensor.matmul(s1[:], ones_c[:, 0:1], xf_a[:, sl], start=True, stop=False)
                nc.tensor.matmul(s1[:], ones_c[0:64, 0:1], xf_b[:, sl], start=False, stop=True)
                sq_a = fb.tile([128, 512], f32, tag="sq")
                sq_b = fb.tile([64, 512], f32, tag="sqb")
                nc.scalar.square(sq_a[:], xf_a[:, sl])
                nc.scalar.square(sq_b[:], xf_b[:, sl])
                s2 = fpp.tile([1, 512], f32, tag="fp")
                nc.tensor.matmul(s2[:], ones_c[:, 0:1], sq_a[:], start=True, stop=False)
                nc.tensor.matmul(s2[:], ones_c[0:64, 0:1], sq_b[:], start=False, stop=True)

                mean = fb.tile([1, 512], f32, tag="mean")
                ex2 = fb.tile([1, 512], f32, tag="ex2")
                nc.vector.tensor_scalar_mul(mean[:], s1[:], 1.0 / C)
                nc.vector.tensor_scalar_mul(ex2[:], s2[:], 1.0 / C)
                var = fb.tile([1, 512], f32, tag="var")
                nc.vector.scalar_tensor_tensor(
                    var[:], mean[:], -1.0, mean[:],
                    op0=mybir.AluOpType.mult, op1=mybir.AluOpType.mult,
                )
                nc.vector.tensor_add(var[:], var[:], ex2[:])
                nc.vector.tensor_scalar_add(var[:], var[:], EPS)
                rcp = fb.tile([1, 512], f32, tag="rcp")
                nc.vector.reciprocal(rcp[:], var[:])
                rstd = fb.tile([1, 512], f32, tag="rstd")
                nc.scalar.sqrt(rstd[:], rcp[:])
                brow = fb.tile([1, 512], f32, tag="brow")
                nc.vector.scalar_tensor_tensor(
                    brow[:], mean[:], -1.0, rstd[:],
                    op0=mybir.AluOpType.mult, op1=mybir.AluOpType.mult,
                )
                ab_ps = fpp.tile([128, 512], f32, tag="fp")
                nc.tensor.matmul(ab_ps[:], ones_r[0:1, :], rstd[:], start=True, stop=True)
                bb_ps = fpp.tile([128, 512], f32, tag="fp")
                nc.tensor.matmul(bb_ps[:], ones_r[0:1, :], brow[:], start=True, stop=True)

                # x_ln in place, then gamma/beta
                nc.vector.tensor_mul(xf_a[:, sl], xf_a[:, sl], ab_ps[:])
                nc.vector.tensor_add(xf_a[:, sl], xf_a[:, sl], bb_ps[:])
                nc.vector.tensor_mul(xf_b[:, sl], xf_b[:, sl], ab_ps[0:64, :])
                nc.vector.tensor_add(xf_b[:, sl], xf_b[:, sl], bb_ps[0:64, :])
                nc.scalar.activation(
                    xf_a[:, sl], xf_a[:, sl], mybir.ActivationFunctionType.Identity,
                    bias=gb_a[:, 1:2], scale=gb_a[:, 0:1],
                )
                nc.scalar.activation(
                    xf_b[:, sl], xf_b[:, sl], mybir.ActivationFunctionType.Identity,
                    bias=gb_b[:, 1:2], scale=gb_b[:, 0:1],
                )

            # ---- fused (1x1 conv + depthwise 3x3) as 9 shifted matmuls ----
            xfa3 = xf_a.rearrange("c (Y X) -> c Y X", X=64)
            xfb3 = xf_b.rearrange("c (Y X) -> c Y X", X=64)
            sec_tiles = (q_s, k_s, v_s)
            for j in range(NT):
                y0 = 8 * j  # first image row of this tile
                for s in range(3):
                    cp = fpp.tile([D, 512], f32, tag="fp")
                    cp3 = cp.rearrange("p (Y X) -> p Y X", X=64)
                    col = (s * 9 + CENTER) * D
                    nc.tensor.matmul(
                        cp[:], wq_a[:, col : col + D],
                        xf_a[:, j * 512 : (j + 1) * 512],
                        start=True, stop=False,
                    )
                    nc.tensor.matmul(
                        cp[:], wq_b[:, col : col + D],
                        xf_b[:, j * 512 : (j + 1) * 512],
                        start=False, stop=False,
                    )
                    for t, (oy, ox) in enumerate(TAPS):
                        if (oy, ox) == (0, 0):
                            continue
                        last = t == len(TAPS) - 1
                        ly0 = max(0, -(y0 + oy))
                        ly1 = min(8, 64 - oy - y0)
                        dx0, dx1 = max(0, -ox), 64 - max(0, ox)
                        col = (s * 9 + t) * D
                        out_ap = cp3[:, ly0:ly1, dx0:dx1]
                        nc.tensor.matmul(
                            out_ap,
                            wq_a[:, col : col + D],
                            xfa3[:, y0 + ly0 + oy : y0 + ly1 + oy, dx0 + ox : dx1 + ox],
                            start=False, stop=False, skip_group_check=True,
                        )
                        nc.tensor.matmul(
                            out_ap,
                            wq_b[:, col : col + D],
                            xfb3[:, y0 + ly0 + oy : y0 + ly1 + oy, dx0 + ox : dx1 + ox],
                            start=False, stop=last, skip_group_check=True,
                        )
                    # bias + copy to sbuf (q/k in f32r)
                    nc.scalar.activation(
                        sec_tiles[s][:, j * 512 : (j + 1) * 512], cp[:],
                        mybir.ActivationFunctionType.Identity,
                        bias=dw_s[:, s : s + 1], scale=1.0,
                    )

            # ---- build vt (v transposed blocks with leading ones column) ----
            for i in range(MB):
                nc.scalar.copy(vt_s[:, i * (D + 1) : i * (D + 1) + 1], ones_c[:, 0:1])
            for i in range(MB):
                vp = fpp.tile([128, D], f32, tag="fp")
                nc.tensor.matmul(
                    vp[:],
                    v_s[:, i * 128 : (i + 1) * 128],
                    id_s[:],
                    start=True, stop=True,
                )
                nc.scalar.copy(vt_s[:, i * (D + 1) + 1 : (i + 1) * (D + 1)], vp[:])

            # residual feed: convert raw x back to f32 into xf (x_ln dead
            # after the conv matmuls); each core folds x/8 into its partial
            nc.scalar.copy(xf_a[:], xh_a[:])
            nc.scalar.copy(xf_b[:], xh_b[:])

            # ---- attention + partial projection ----
            for j in range(NT):
                o2 = acp.tile([D + 1, 512], f32, tag="acc")
                qv = q_s[:, j * 512 : (j + 1) * 512]
                for g in range(NT):
                    sp = spp.tile([128, 2048], f32, tag="sp")
                    for i in range(4):
                        m = 4 * g + i
                        nc.tensor.matmul(
                            sp[:, i * 512 : (i + 1) * 512],
                            k_s[:, m * 128 : (m + 1) * 128],
                            qv,
                            start=True,
                            stop=True,
                        )
                    pt = sb.tile([128, 2048], f32r, tag="pt")
                    nc.scalar.activation(
                        pt[:], sp[:], mybir.ActivationFunctionType.Exp,
                        scale=tpb[:, 0:1],
                    )
                    for i in range(4):
                        m = 4 * g + i
                        nc.tensor.matmul(
                            o2[:],
                            vt_s[:, m * (D + 1) : (m + 1) * (D + 1)],
                            pt[:, i * 512 : (i + 1) * 512],
                            start=(m == 0),
                            stop=(m == MB - 1),
                        )
                u = sb.tile([D + 1, 512], f32, tag="u")
                nc.vector.tensor_copy(u[:], o2[:])
                r = sb.tile([1, 512], f32, tag="r")
                nc.vector.reciprocal(r[:], u[0:1, :])
                rb = acp.tile([D + 1, 512], f32, tag="acc")
                nc.tensor.matmul(
                    rb[:], ones_r[0:1, 0 : D + 1], r[:], start=True, stop=True
                )
                un = sb.tile([D + 1, 512], f32, tag="un")
                nc.vector.tensor_mul(un[:], u[:], rb[:])
                sl = slice(j * 512, (j + 1) * 512)
                ya_ps = acp.tile([128, 512], f32, tag="acc")
                nc.tensor.matmul(ya_ps[:], wp_s[:, 0:128], un[:], start=True, stop=True)
                # y_partial = proj + x/8 (residual folded in, rank-free)
                nc.vector.scalar_tensor_tensor(
                    y_a[:, sl], xf_a[:, sl], 0.125, ya_ps[:],
                    op0=mybir.AluOpType.mult, op1=mybir.AluOpType.add,
                )
                yb_ps = acp.tile([64, 512], f32, tag="acc")
                nc.tensor.matmul(yb_ps[:], wp_s[:, 128:C], un[:], start=True, stop=True)
                nc.vector.scalar_tensor_tensor(
                    y_b[:, sl], xf_b[:, sl], 0.125, yb_ps[:],
                    op0=mybir.AluOpType.mult, op1=mybir.AluOpType.add,
                )

            # ---- ReduceScatter partials: core c receives channel slice c ----
            nc.gpsimd.dma_start(out=rs_in[0:128, :], in_=y_a[:])
            nc.gpsimd.dma_start(out=rs_in[128:C, :], in_=y_b[:])
            nc.gpsimd.collective_compute(
                "ReduceScatter",
                mybir.AluOpType.add,
                replica_groups=RG,
                ins=[rs_in.opt()],
                outs=[rs_out.opt()],
            )
            yr = pp.tile([D, N], f32, tag="vs")  # reuse v_s space (dead)
            nc.sync.dma_start(out=yr[:], in_=rs_out[:])
            y16 = pp.tile([D, N], f16, tag="x16")  # reuse x16 space (dead)
            nc.vector.tensor_copy(y16[:], yr[:])
            nc.sync.dma_start(out=y_d[:], in_=y16[:])
    nc.compile()
    return nc


def _make_runner():
    """Build the bass program once and a cached jit dispatcher around it,
    mirroring concourse.bass2jax.run_bass_via_pjrt but reusable per call."""
    if "runner" in _cache:
        return _cache["runner"]
    import jax
    import jax.numpy as jnp
    from jax.sharding import Mesh, PartitionSpec as P, NamedSharding
    try:
        from jax import shard_map

        def _shard_map(f, mesh, in_specs, out_specs):
            return shard_map(f, mesh=mesh, in_specs=in_specs, out_specs=out_specs,
                             check_vma=False)
    except ImportError:
        from jax.experimental.shard_map import shard_map

        def _shard_map(f, mesh, in_specs, out_specs):
            return shard_map(f, mesh=mesh, in_specs=in_specs, out_specs=out_specs,
                             check_rep=False)
    from concourse import bass2jax

    nc = _build_program()
    bass2jax.install_neuronx_cc_hook()
    assert nc.dbg_addr is None
    partition_name = nc.partition_id_tensor.name if nc.partition_id_tensor else None

    in_names = []
    out_names = []
    out_avals = []
    for alloc in nc.m.functions[0].allocations:
        if not isinstance(alloc, mybir.MemoryLocationSet):
            continue
        name = alloc.memorylocations[0].name
        if alloc.kind == "ExternalInput":
            if name != partition_name:
                in_names.append(name)
        elif alloc.kind == "ExternalOutput":
            shape = tuple(alloc.tensor_shape)
            dtype = mybir.dt.np(alloc.dtype)
            out_avals.append(jax.core.ShapedArray(shape, dtype))
            out_names.append(name)
    n_params = len(in_names)
    n_outs = len(out_names)
    # no donated zero buffers: the kernel writes every output element, so
    # uninitialized custom-call result buffers are fine
    all_names = list(in_names)
    if partition_name is not None:
        all_names.append(partition_name)

    def _body(*args):
        operands = list(args)
        if partition_name is not None:
            operands.append(bass2jax.partition_id_tensor())
        outs = bass2jax._bass_exec_p.bind(
            *operands,
            out_avals=tuple(out_avals),
            in_names=tuple(all_names),
            out_names=tuple(out_names),
            lowering_input_output_aliases=(),
            sim_require_finite=True,
            sim_require_nnan=True,
            nc=nc,
        )
        return tuple(outs)

    devices = jax.devices()[:8]
    mesh = Mesh(np.asarray(devices), ("core",))
    sharding = NamedSharding(mesh, P("core"))
    in_specs = (P("core"),) * n_params
    out_specs = (P("core"),) * n_outs
    sharded = jax.jit(
        _shard_map(_body, mesh, in_specs, out_specs),
        keep_unused=True,
    )
    runner = {
        "sharded": sharded,
        "in_names": in_names,
        "out_names": out_names,
        "out_avals": out_avals,
        "sharding": sharding,
        "device_put": jax.device_put,
    }
    _cache["runner"] = runner
    return runner


def _weights_device(runner, w_qkv, w_dw, b_dw, w_proj, gamma, beta, temperature):
    """Upload per-core weight arrays once; reuse across calls when unchanged."""
    key = "weights"
    raw = (w_qkv, w_dw, b_dw, w_proj, gamma, beta, temperature)
    if key in _cache:
        saved_raw, dev = _cache[key]
        if all(np.array_equal(a, b) for a, b in zip(saved_raw, raw)):
            return dev
    wq_l, dw_l, wp_l, gb_l, tp_l, id_l = [], [], [], [], [], []
    eye = np.eye(D, dtype=np.float32)
    gb = np.stack([gamma, beta], axis=1).astype(np.float32)  # [C,2]
    temp = temperature.reshape(HEADS)
    taps9 = [(dy + 1) * 3 + (dx + 1) for (dy, dx) in TAPS]  # tap order -> w_dw idx
    for h in range(HEADS):
        sl = slice(h * D, (h + 1) * D)
        wq = np.zeros((C, 27 * D), np.float32)
        dw = np.zeros((D, 3), np.float32)
        for s, base in enumerate((h * D, C + h * D, 2 * C + h * D)):
            wsec = w_qkv[base : base + D]  # [D, C]
            dtap = w_dw[base : base + D, 0].reshape(D, 9)  # [D, 9] (dy,dx) row-major
            for t, t9 in enumerate(taps9):
                colb = (s * 9 + t) * D
                wq[:, colb : colb + D] = (wsec * dtap[:, t9 : t9 + 1]).T
            dw[:, s] = b_dw[base : base + D]
        wq_l.append(wq)
        dw_l.append(dw)
        wp = np.zeros((D + 1, C), np.float32)
        wp[1:, :] = w_proj[:, sl].T
        wp_l.append(wp)
        gb_l.append(gb)
        tp_l.append(temp[h : h + 1].reshape(1, 1).astype(np.float32))
        id_l.append(eye)
    by_name = {
        "wq": np.concatenate(wq_l, axis=0),
        "dw": np.concatenate(dw_l, axis=0),
        "wp": np.concatenate(wp_l, axis=0),
        "gb": np.concatenate(gb_l, axis=0),
        "tp": np.concatenate(tp_l, axis=0),
        "id24": np.concatenate(id_l, axis=0),
    }
    dev = {k: runner["device_put"](v, runner["sharding"]) for k, v in by_name.items()}
    for v in dev.values():
        v.block_until_ready()
    saved_raw = tuple(np.array(a, copy=True) for a in raw)
    _cache[key] = (saved_raw, dev)
    return dev


def kernel(x, gamma, beta, w_qkv, w_dw, b_dw, w_proj, temperature):
    x = np.asarray(x, dtype=np.float32)
    gamma = np.asarray(gamma, np.float32)
    beta = np.asarray(beta, np.float32)
    w_qkv = np.asarray(w_qkv, np.float32)
    w_dw = np.asarray(w_dw, np.float32)
    b_dw = np.asarray(b_dw, np.float32)
    w_proj = np.asarray(w_proj, np.float32)
    temperature = np.asarray(temperature, np.float32)

    runner = _make_runner()
    dev = _weights_device(runner, w_qkv, w_dw, b_dw, w_proj, gamma, beta, temperature)

    # channel-sharded upload: core c gets channels 24c..24c+24 (no transpose)
    xs = _cast(x.reshape(C, N), np.float16)

    args = []
    for name in runner["in_names"]:
        args.append(xs if name == "x" else dev[name])
    outs = runner["sharded"](*args)
    # core c returns channels 24c..24c+24: concat along axis 0 is y directly
    y16 = np.asarray(outs[0])  # [192, 4096] f16
    y = _cast(y16, np.float32).reshape(1, C, 64, 64)
    return y

